# revision 27
# baseline (speedup 1.0000x reference)
"""Trainium2 Bass kernel for nn_DTFDynamicLayer (moe_routing dynamic-token
transformer layer), SPMD across 8 NeuronCores.

kernel(**inputs) takes FULL unsharded numpy inputs and returns the FULL
[B,T,D] output. Sharding strategy:
  - router (scores/topk/positions): token-sharded (512 tokens/core) + tiny
    AllGathers of scores/mask/positions
  - dense Qwen2 block head-sharded: each core computes Q/K/V/attention for
    its 2 heads over the FULL packed sequence S=2048; attention outputs are
    exchanged with a 1MB AllToAll so each core does the full O-projection
    for its own 256 packed slots locally (residual + rmsnorm local)
  - MLP tensor-parallel over intermediate dim (704/core) over full S,
    partials combined with ReduceScatter back to own slots
  - dense-block matmuls run in bf16 (fp32 PSUM accumulation); router stays
    fp32 end-to-end so top-k selection is exact
"""
from contextlib import ExitStack

import numpy as np
import ml_dtypes

import concourse.bass as bass
import concourse.mybir as mybir
import concourse.tile as tile
from concourse import bacc
from concourse.bass_utils import run_bass_kernel_spmd
from concourse.masks import make_identity

B, T, D = 2, 2048, 2048
H, HD = 16, 128
I = 5632
EPS = 1e-6
NC = 8
BT = B * T
TOKS = BT // NC          # 512 router tokens per core
K = T // 2               # 1024 selected per batch row
S = B * K                # 2048 packed tokens
SB = S // NC             # 256 packed slots per core
HPC = H // NC            # 2 heads per core
ICOL = I // NC           # 704
DC = D // 128            # 16
SC = S // 128            # 16 slot chunks of 128
SCALE = 1.0 / float(np.sqrt(HD))
IC_CH = [128] * 5 + [ICOL - 5 * 128]   # I-col chunks per core: 5x128 + 64

F32 = mybir.dt.float32
BF16 = mybir.dt.bfloat16
I32 = mybir.dt.int32
AF = mybir.ActivationFunctionType
OP = mybir.AluOpType
P = 128
MC = 512                 # moving-dim chunk for matmuls

_NC_CACHE = {}


def _rmsnorm_now(nc, pool, x, out, epst):
    """out = x * rsqrt(mean(x^2)+eps)  ([128, D] token-major, no weight)."""
    sq = pool.tile([P, D], F32, name="rn_sq")
    ssq = pool.tile([P, 1], F32, name="rn_ssq")
    nc.scalar.activation(sq[:], x[:], AF.Square, accum_out=ssq[:])
    rt = pool.tile([P, 1], F32, name="rn_rt")
    nc.scalar.activation(rt[:], ssq[:], AF.Sqrt, scale=1.0 / D,
                         bias=epst[:, :1])
    rec = pool.tile([P, 1], F32, name="rn_rec")
    nc.vector.reciprocal(rec[:], rt[:])
    nc.scalar.activation(out[:], x[:], AF.Copy, scale=rec[:, :1])


def build(phases="full", gate1=False):
    nc = bacc.Bacc(None, target_bir_lowering=False)
    _build(nc, phases, gate1)
    nc.finalize()
    return nc


def _build(nc, phases, gate1=False):
    dp = nc.declare_dram_parameter
    orig_s = dp("orig_s", [TOKS, D], F32, isOutput=False)
    post_s = dp("post_s", [TOKS, D], F32, isOutput=False)
    prior_s = dp("prior_s", [TOKS, D], F32, isOutput=False)
    hidden = dp("hidden", [BT, D], F32, isOutput=False)
    cossin = dp("cossin", [BT, 2 * HD], F32, isOutput=False)
    qw_s = dp("qw_s", [D, HPC * HD], BF16, isOutput=False)
    kw_s = dp("kw_s", [D, HPC * HD], BF16, isOutput=False)
    vw_s = dp("vw_s", [D, HPC * HD], BF16, isOutput=False)
    qb_s = dp("qb_s", [HPC * HD, 1], F32, isOutput=False)
    kb_s = dp("kb_s", [HPC * HD, 1], F32, isOutput=False)
    vb_s = dp("vb_s", [HPC * HD, 1], F32, isOutput=False)
    ow = dp("ow", [H * HD, D], BF16, isOutput=False)
    ln1w = dp("ln1w", [D, 1], F32, isOutput=False)
    ln2w = dp("ln2w", [D, 1], F32, isOutput=False)
    gatew_s = dp("gatew_s", [D, ICOL], BF16, isOutput=False)
    upw_s = dp("upw_s", [D, ICOL], BF16, isOutput=False)
    downw_s = dp("downw_s", [ICOL, D], BF16, isOutput=False)
    # cconst: [beta_cu, beta_ce, beta_ce*ce_off, i0(=c*SB), unused,
    #          unused, i0row(=(c%4)*TOKS), b(=c//4)]
    cconst = dp("cconst", [1, 8], F32, isOutput=False)

    upd_out = dp("upd_out", [SB, D], F32, isOutput=True)
    selidx_out = dp("selidx_out", [SB, 1], I32, isOutput=True)
    dbg = dp("dbg", [P, 16], F32, isOutput=True)

    RG = [list(range(NC))]

    with tile.TileContext(nc) as tc, ExitStack() as es:
        # -------- DRAM internals (pool tiles => dep tracking) --------
        dr = es.enter_context(tc.tile_pool(name="dram", bufs=1, space="DRAM"))

        def dtile(name, shape, dtype=F32, shared=False):
            return dr.tile(shape, dtype, name=name,
                           addr_space="Shared" if shared else "Local")

        sc_in = dtile("sc_in", [TOKS, 1])
        sc_all = dtile("sc_all", [BT, 1], shared=True)
        mk_in = dtile("mk_in", [TOKS, 1])
        mk_all = dtile("mk_all", [BT, 1], shared=True)
        ps_in = dtile("ps_in", [TOKS, 1])
        ps_all = dtile("ps_all", [BT, 1], shared=True)
        h1t_in = dtile("h1t_in", [D, SB], BF16)
        h1t_all = dtile("h1t_all", [NC * D, SB], BF16, shared=True)
        o_a2a = dtile("o_a2a", [HPC * HD * NC, SB], BF16)
        o_all = dtile("o_all", [H * HD, SB], BF16)
        h2t_in = dtile("h2t_in", [D, SB], BF16)
        h2t_all = dtile("h2t_all", [NC * D, SB], BF16, shared=True)
        mlp_in = dtile("mlp_in", [S, D], BF16)
        mlp_rs = dtile("mlp_rs", [SB, D], BF16)

        # -------- persistent SBUF --------
        pers = es.enter_context(tc.tile_pool(name="pers", bufs=1))
        ident = pers.tile([P, P], F32)
        make_identity(nc, ident[:])
        cc_sb = pers.tile([1, 8], F32)
        nc.sync.dma_start(out=cc_sb[:], in_=cconst[:])
        ccb = pers.tile([P, 8], F32)
        nc.gpsimd.partition_broadcast(ccb[:], cc_sb[:])
        col_bcu = ccb[:, 0:1]
        col_bce = ccb[:, 1:2]
        col_ceo = ccb[:, 2:3]
        col_i0 = ccb[:, 3:4]
        col_i0row = ccb[:, 6:7]
        col_b = ccb[:, 7:8]
        ones_bf = pers.tile([P, 1], BF16)
        nc.vector.memset(ones_bf[:], 1.0)
        epst = pers.tile([P, 1], F32)
        nc.vector.memset(epst[:], EPS)
        iota_jmp = pers.tile([P, T], F32)      # value = j - p
        _it2 = pers.tile([P, T], I32)
        nc.gpsimd.iota(_it2[:], pattern=[[1, T]], base=0, channel_multiplier=-1)
        nc.vector.tensor_copy(iota_jmp[:], _it2[:])
        # causal masks for diagonal 128x512 blocks: keep iff p - f <= -128*l
        iota_pf = pers.tile([P, MC], F32)      # value = p - f
        _it = pers.tile([P, MC], I32)
        nc.gpsimd.iota(_it[:], pattern=[[-1, MC]], base=0, channel_multiplier=1)
        nc.vector.tensor_copy(iota_pf[:], _it[:])
        masks = pers.tile([P, 4, MC], BF16)
        for l in range(4):
            nc.vector.tensor_scalar(masks[:, l, :], iota_pf[:],
                                    float(-128 * l), None, op0=OP.is_le)
        lnw_cols = pers.tile([P, 2 * DC], F32)  # [:, 0:16]=ln1, [:,16:32]=ln2
        nc.sync.dma_start(out=lnw_cols[:, 0:DC],
                          in_=ln1w.rearrange("(d p) one -> p d one", p=P))
        nc.sync.dma_start(out=lnw_cols[:, DC:2 * DC],
                          in_=ln2w.rearrange("(d p) one -> p d one", p=P))
        dbg_t = pers.tile([P, 16], F32)
        nc.vector.memset(dbg_t[:], 0.0)

        s_cols = [pers.tile([P, 1], F32, name=f"s_col{t}") for t in range(4)]
        m_cols = [pers.tile([P, 1], F32, name=f"m_col{t}") for t in range(4)]
        p_cols = [pers.tile([P, 1], F32, name=f"p_col{t}") for t in range(4)]

        # -------- weight prefetch (persists through the phases using it) ----
        pw = es.enter_context(tc.tile_pool(name="pw", bufs=1))
        # qkv projection weights for own heads: per proj, 16 d-tiles [128,256]
        wq = pw.tile([P, DC, HPC * HD], BF16)
        wk = pw.tile([P, DC, HPC * HD], BF16)
        wv = pw.tile([P, DC, HPC * HD], BF16)
        for (src, dst) in ((qw_s, wq), (kw_s, wk), (vw_s, wv)):
            for d in range(DC):
                nc.sync.dma_start(out=dst[:, d, :],
                                  in_=src[d * P:(d + 1) * P, :])
        qkvb = pw.tile([P, 3 * HPC], F32)   # q0 q1 k0 k1 v0 v1 bias columns
        for j, src in enumerate((qb_s, kb_s, vb_s)):
            nc.sync.dma_start(
                out=qkvb[:, j * HPC:(j + 1) * HPC],
                in_=src.rearrange("(h p) one -> p h one", p=P))


        # ============ Phase R1: scores for own 512 tokens ============
        with tc.tile_pool(name="router", bufs=3) as rp:
            for t in range(4):
                at = rp.tile([P, D], F32, name="r_at")
                bt = rp.tile([P, D], F32, name="r_bt")
                ct = rp.tile([P, D], F32, name="r_ct")
                nc.sync.dma_start(out=at[:], in_=orig_s[t * P:(t + 1) * P, :])
                nc.sync.dma_start(out=bt[:], in_=post_s[t * P:(t + 1) * P, :])
                nc.sync.dma_start(out=ct[:], in_=prior_s[t * P:(t + 1) * P, :])
                cu = rp.tile([P, 1], F32, name="cu")
                ce = rp.tile([P, 1], F32, name="ce")
                for (x_ap, y_ap, dst) in ((at, bt, cu), (bt, ct, ce)):
                    df = rp.tile([P, D], F32, name="r_df")
                    nc.vector.tensor_sub(df[:], x_ap[:], y_ap[:])
                    sq = rp.tile([P, D], F32, name="r_sq")
                    ssq = rp.tile([P, 1], F32, name="r_ssq")
                    nc.scalar.activation(sq[:], df[:], AF.Square,
                                         accum_out=ssq[:])
                    nc.scalar.activation(dst[:], ssq[:], AF.Sqrt)
                t1 = rp.tile([P, 1], F32, name="r_t1")
                nc.vector.tensor_scalar(t1[:], cu[:], col_bcu, None,
                                        op0=OP.mult)
                nc.vector.scalar_tensor_tensor(
                    s_cols[t][:], in0=ce[:], scalar=col_bce, in1=t1[:],
                    op0=OP.mult, op1=OP.add)
                nc.vector.tensor_scalar(s_cols[t][:], s_cols[t][:], col_ceo,
                                        None, op0=OP.add)
            sc_flat = rp.tile([P, 4], F32, name="scflat")
            for t in range(4):
                nc.vector.tensor_copy(sc_flat[:, t:t + 1], s_cols[t][:])
            nc.sync.dma_start(
                out=sc_in.rearrange("(t p) one -> p t one", p=P),
                in_=sc_flat[:])
        nc.gpsimd.collective_compute("AllGather", OP.bypass, replica_groups=RG,
                                     ins=[sc_in[:]], outs=[sc_all[:]])

        # ============ Phase R2: rank -> mask for own tokens ============
        # rank_i = #{j: s_j>s_i} + #{j<i: s_j==s_i} = (T - sum(le)) + sum(eq*jlt)
        # mask = rank <= K-1  <=>  acc = sum(le) - sum(eq*jlt) >= T-K+1
        with tc.tile_pool(name="rank1", bufs=1) as rp1, \
             tc.tile_pool(name="rank", bufs=2) as rp:
            sbr = rp1.tile([P, T], F32, name="sbr")
            _row_select_bcast(nc, rp1, sc_all, col_b, sbr)
            for t in range(4):
                jlt = rp.tile([P, T], F32, name="k_jlt")
                rhs = rp.tile([P, 1], F32, name="k_rhs")
                nc.vector.tensor_scalar(rhs[:], col_i0row, float(t * P - 1),
                                        None, op0=OP.add)
                nc.vector.tensor_scalar(jlt[:], iota_jmp[:], rhs[:, :1], None,
                                        op0=OP.is_le)
                le = rp.tile([P, T], F32, name="k_le")
                nc.vector.tensor_scalar(le[:], sbr[:], s_cols[t][:, :1], None,
                                        op0=OP.is_le)
                eq = rp.tile([P, T], F32, name="k_eq")
                nc.vector.tensor_scalar(eq[:], sbr[:], s_cols[t][:, :1], None,
                                        op0=OP.is_equal)
                nc.vector.tensor_mul(eq[:], eq[:], jlt[:])
                nc.vector.tensor_sub(le[:], le[:], eq[:])
                acc = rp.tile([P, 1], F32, name="k_acc")
                nc.vector.tensor_reduce(acc[:], le[:],
                                        axis=mybir.AxisListType.X, op=OP.add)
                # mask = acc >= T-K+1  <=>  (-acc) <= -(T-K+1)
                nacc = rp.tile([P, 1], F32, name="k_nacc")
                nc.vector.tensor_scalar_mul(nacc[:], acc[:], -1.0)
                nc.vector.tensor_scalar(m_cols[t][:], nacc[:],
                                        float(-(T - K + 1)), None,
                                        op0=OP.is_le)
            mflat = rp.tile([P, 4], F32, name="mflat")
            for t in range(4):
                nc.vector.tensor_copy(mflat[:, t:t + 1], m_cols[t][:])
            nc.sync.dma_start(
                out=mk_in.rearrange("(t p) one -> p t one", p=P), in_=mflat[:])
        nc.gpsimd.collective_compute("AllGather", OP.bypass, replica_groups=RG,
                                     ins=[mk_in[:]], outs=[mk_all[:]])

        # ============ Phase R3: positions ============
        with tc.tile_pool(name="pos1", bufs=1) as rp1, \
             tc.tile_pool(name="pos", bufs=2) as rp:
            mbr = rp1.tile([P, T], F32, name="mbr")
            _row_select_bcast(nc, rp1, mk_all, col_b, mbr)
            for t in range(4):
                jlt = rp.tile([P, T], F32, name="p_jlt")
                rhs = rp.tile([P, 1], F32, name="p_rhs")
                nc.vector.tensor_scalar(rhs[:], col_i0row, float(t * P - 1),
                                        None, op0=OP.add)
                nc.vector.tensor_scalar(jlt[:], iota_jmp[:], rhs[:, :1], None,
                                        op0=OP.is_le)
                mj = rp.tile([P, T], F32, name="p_mj")
                nc.vector.tensor_mul(mj[:], mbr[:], jlt[:])
                nc.vector.tensor_reduce(p_cols[t][:], mj[:],
                                        axis=mybir.AxisListType.X, op=OP.add)
            pflat = rp.tile([P, 4], F32, name="pflat")
            for t in range(4):
                nc.vector.tensor_copy(pflat[:, t:t + 1], p_cols[t][:])
            nc.sync.dma_start(
                out=ps_in.rearrange("(t p) one -> p t one", p=P), in_=pflat[:])
        nc.gpsimd.collective_compute("AllGather", OP.bypass, replica_groups=RG,
                                     ins=[ps_in[:]], outs=[ps_all[:]])

        # ======= Phase INV: slot -> flat row map via counting =======
        # row(s) = #{j in batch row: inc_j <= s_local} (+ b*T), where
        # inc = inclusive cumsum of mask = pos + mask.
        gpL = es.enter_context(tc.tile_pool(name="gpL", bufs=1))   # long-lived
        rows_i = gpL.tile([P, SC], I32)
        own_rows = [gpL.tile([P, 1], I32, name=f"orow{h}") for h in range(2)]
        _slotf = gpL.tile([P, 2], F32)
        with tc.tile_pool(name="inv1", bufs=1) as iv, \
             tc.tile_pool(name="inv", bufs=2) as ivs:
            incb = [iv.tile([P, T], F32, name=f"incb{b}") for b in range(2)]
            vps = ps_all.rearrange("(a t) one -> a (t one)", a=2)
            vmk = mk_all.rearrange("(a t) one -> a (t one)", a=2)
            for b in range(2):
                pr = ivs.tile([1, T], F32, name="i_pr")
                mr = ivs.tile([1, T], F32, name="i_mr")
                nc.sync.dma_start(out=pr[:], in_=vps[b:b + 1, :])
                nc.sync.dma_start(out=mr[:], in_=vmk[b:b + 1, :])
                inc1 = ivs.tile([1, T], F32, name="i_inc")
                nc.vector.tensor_add(inc1[:], pr[:], mr[:])
                nc.gpsimd.partition_broadcast(incb[b][:], inc1[:])
            sid_i = iv.tile([P, 8], I32)
            nc.gpsimd.iota(sid_i[:], pattern=[[P, 8]], base=0,
                           channel_multiplier=1)
            sid = iv.tile([P, 8], F32)
            nc.vector.tensor_copy(sid[:], sid_i[:])
            rows_f = iv.tile([P, SC], F32)
            for g in range(SC):
                b, gl = g // 8, g % 8
                tmp = ivs.tile([P, T], F32, name="i_tmp", tag="i_tmp")
                nc.vector.tensor_scalar(tmp[:], incb[b][:],
                                        sid[:, gl:gl + 1], None,
                                        op0=OP.is_le)
                nc.vector.tensor_reduce(rows_f[:, g:g + 1], tmp[:],
                                        axis=mybir.AxisListType.X, op=OP.add)
            nc.vector.tensor_scalar(rows_f[:, 8:SC], rows_f[:, 8:SC],
                                    float(T), None, op0=OP.add)
            nc.vector.tensor_copy(rows_i[:], rows_f[:])
            # own slots: local id = i0 + half*128 + p - b*K
            _si = iv.tile([P, 2], I32)
            for half in range(2):
                nc.gpsimd.iota(_si[:, half:half + 1], pattern=[[0, 1]],
                               base=half * P, channel_multiplier=1)
            nc.vector.tensor_copy(_slotf[:], _si[:])
            for half in range(2):
                nc.vector.tensor_scalar(_slotf[:, half:half + 1],
                                        _slotf[:, half:half + 1], col_i0,
                                        None, op0=OP.add)
            incsel = iv.tile([P, T], F32)
            dfi = ivs.tile([P, T], F32, name="i_dfi")
            nc.vector.tensor_sub(dfi[:], incb[1][:], incb[0][:])
            nc.vector.scalar_tensor_tensor(incsel[:], in0=dfi[:],
                                           scalar=col_b, in1=incb[0][:],
                                           op0=OP.mult, op1=OP.add)
            bk = iv.tile([P, 1], F32)
            nc.vector.tensor_scalar(bk[:], col_b, float(K), None, op0=OP.mult)
            btt = iv.tile([P, 1], F32)
            nc.vector.tensor_scalar(btt[:], col_b, float(T), None,
                                    op0=OP.mult)
            for half in range(2):
                ls = iv.tile([P, 1], F32, name=f"ls{half}")
                nc.vector.tensor_sub(ls[:], _slotf[:, half:half + 1], bk[:])
                tmp = ivs.tile([P, T], F32, name="i_tmp2", tag="i_tmp2")
                orf = iv.tile([P, 1], F32, name=f"orf{half}")
                nc.vector.tensor_scalar(tmp[:], incsel[:], ls[:, :1], None,
                                        op0=OP.is_le)
                nc.vector.tensor_reduce(orf[:], tmp[:],
                                        axis=mybir.AxisListType.X, op=OP.add)
                nc.vector.tensor_add(orf[:], orf[:], btt[:])
                nc.vector.tensor_copy(own_rows[half][:], orf[:])

        # ============ Phase G: gathers ============
        selh = []
        gate_g = []
        for half in range(2):
            orow = own_rows[half]
            sh = gpL.tile([P, D], F32, name=f"selh{half}")
            nc.gpsimd.indirect_dma_start(
                out=sh[:], out_offset=None, in_=hidden[:],
                in_offset=bass.IndirectOffsetOnAxis(ap=orow[:, :1], axis=0),
                bounds_check=BT - 1, oob_is_err=False)
            selh.append(sh)
            ssc = gpL.tile([P, 1], F32, name=f"ssc{half}")
            nc.gpsimd.indirect_dma_start(
                out=ssc[:], out_offset=None, in_=sc_all[:],
                in_offset=bass.IndirectOffsetOnAxis(ap=orow[:, :1], axis=0))
            gg = gpL.tile([P, 1], F32, name=f"gate{half}")
            if gate1:
                nc.vector.memset(gg[:], 1.0)   # debug: bypass gate
            else:
                nc.scalar.activation(gg[:], ssc[:], AF.Sigmoid)
            gate_g.append(gg)
        x1 = [gpL.tile([P, D], F32, name=f"x1_{i}") for i in range(2)]

        if phases == "router":
            with tc.tile_pool(name="rfin", bufs=2) as fp:
                for half in range(2):
                    nc.sync.dma_start(
                        out=upd_out[half * P:(half + 1) * P, :],
                        in_=selh[half][:])
                    nc.sync.dma_start(
                        out=selidx_out[half * P:(half + 1) * P, :],
                        in_=own_rows[half][:])
                nc.vector.tensor_copy(dbg_t[:, 4:5], p_cols[0][:])
                nc.vector.tensor_copy(dbg_t[:, 5:6], gate_g[0][:])
                nc.sync.dma_start(out=dbg[:], in_=dbg_t[:])
            return

        # attention-era pool: cos/sin, q/k/v, o (LIFO: opened now, closed
        # after the o_a2a DMA)
        esA = ExitStack()
        gpA = esA.enter_context(tc.tile_pool(name="gpA", bufs=1))
        cosT = gpA.tile([P, S], F32)
        sinm = gpA.tile([P, S], F32)     # rows 0:64 pre-negated
        q_sb = [gpA.tile([P, S], BF16, name=f"q_sb{h}") for h in range(HPC)]
        k_sb = [gpA.tile([P, S], BF16, name=f"k_sb{h}") for h in range(HPC)]
        # v stored slot-major [key-in-chunk, jc, hd] (AV stationary needs
        # contraction over keys on the partition dim)
        v_sb = [gpA.tile([P, SC, P], BF16, name=f"v_sb{h}") for h in range(HPC)]
        o_sb = [gpA.tile([P, S], BF16, name=f"o_sb{h}") for h in range(HPC)]

        # cos/sin gather+transpose for all S slots
        with tc.tile_pool(name="cso", bufs=3) as cp, \
             tc.tile_pool(name="csop", bufs=4, space="PSUM") as cpp:
            for t in range(SC):
                g = cp.tile([P, 2 * HD], F32, name="cs_g")
                nc.gpsimd.indirect_dma_start(
                    out=g[:], out_offset=None, in_=cossin[:],
                    in_offset=bass.IndirectOffsetOnAxis(
                        ap=rows_i[:, t:t + 1], axis=0))
                sl = slice(t * P, (t + 1) * P)
                ptc = cpp.tile([P, P], F32, space="PSUM", name="cs_pc")
                nc.tensor.transpose(ptc[:], g[:, 0:HD], ident[:])
                nc.vector.tensor_copy(cosT[:, sl], ptc[:])
                pts = cpp.tile([P, P], F32, space="PSUM", name="cs_ps")
                nc.tensor.transpose(pts[:], g[:, HD:2 * HD], ident[:])
                nc.scalar.activation(sinm[0:64, sl], pts[0:64, :], AF.Copy,
                                     scale=-1.0)
                nc.scalar.activation(sinm[64:P, sl], pts[64:P, :], AF.Copy)

        # ============ Phase N1: h1 = rmsnorm(selh)*ln1, transposed; AG ======
        with tc.tile_pool(name="n1", bufs=2) as np_, \
             tc.tile_pool(name="n1p", bufs=4, space="PSUM") as npp:
            h1T_own = np_.tile([P, DC, SB], BF16, name="h1T_own")
            for half in range(2):
                h1 = np_.tile([P, D], F32, name="h1")
                _rmsnorm_now(nc, np_, selh[half], h1, epst)
                for d in range(DC):
                    pt = npp.tile([P, P], F32, space="PSUM", name="n1_tp")
                    nc.tensor.transpose(pt[:], h1[:, d * P:(d + 1) * P],
                                        ident[:])
                    nc.scalar.activation(
                        h1T_own[:, d, half * P:(half + 1) * P], pt[:],
                        AF.Copy, scale=lnw_cols[:, d:d + 1])
            for d in range(DC):
                nc.sync.dma_start(out=h1t_in[d * P:(d + 1) * P, :],
                                  in_=h1T_own[:, d, :])
        nc.gpsimd.collective_compute("AllGather", OP.bypass, replica_groups=RG,
                                     ins=[h1t_in[:]], outs=[h1t_all[:]])

        # view of h1t_all rows (c dd p) -> [dd, p, c, s]
        h1t_v = h1t_all.rearrange("(c dd p) s -> dd p c s", c=NC, p=P)

        # ============ Phase QKV (own 2 heads over full S) ============
        for half in range(2):
            hs = slice(half * (S // 2), (half + 1) * (S // 2))
            with tc.tile_pool(name="qkvr", bufs=1) as qr, \
                 tc.tile_pool(name="qkv", bufs=2) as qp, \
                 tc.tile_pool(name="qkvp", bufs=2, space="PSUM") as qpp:
                rhs = [qr.tile([P, S // 2], BF16, name=f"rhs{d}",
                               tag=f"rhs{d}") for d in range(DC)]
                for d in range(DC):
                    nc.sync.dma_start(
                        out=rhs[d][:],
                        in_=h1t_v[d][:, 4 * half:4 * half + 4, :])
                for (wt, bcol0, dsts, rope) in (
                        (wq, 0, q_sb, True), (wk, HPC, k_sb, True),
                        (wv, 2 * HPC, v_sb, False)):
                    for hh in range(HPC):
                        ps = [qpp.tile([P, MC], F32, space="PSUM",
                                       name=f"qkv_ps{j}", tag=f"qkv_ps{j}")
                              for j in range(2)]
                        for d in range(DC):
                            for j in range(2):
                                nc.tensor.matmul(
                                    ps[j][:],
                                    wt[:, d, hh * HD:(hh + 1) * HD],
                                    rhs[d][:, j * MC:(j + 1) * MC],
                                    start=(d == 0), stop=(d == DC - 1),
                                    skip_group_check=True)
                        for j in range(2):
                            cs = slice(half * (S // 2) + j * MC,
                                       half * (S // 2) + (j + 1) * MC)
                            if not rope:
                                # V: bias then transpose each 128-slot chunk
                                vh = qp.tile([P, MC], F32, name="vh",
                                             tag="vh")
                                nc.scalar.activation(
                                    vh[:], ps[j][:], AF.Identity,
                                    bias=qkvb[:, bcol0 + hh:bcol0 + hh + 1])
                                for c2 in range(4):
                                    jcg = half * 8 + j * 4 + c2
                                    vt = qpp.tile([P, P], F32, space="PSUM",
                                                  name="v_tp", tag="v_tp")
                                    nc.tensor.transpose(
                                        vt[:], vh[:, c2 * P:(c2 + 1) * P],
                                        ident[:])
                                    nc.vector.tensor_copy(
                                        dsts[hh][:, jcg, :], vt[:])
                            else:
                                qh = qp.tile([P, MC], F32, name="qh", tag="qh")
                                nc.scalar.activation(
                                    qh[:], ps[j][:], AF.Identity,
                                    bias=qkvb[:, bcol0 + hh:bcol0 + hh + 1])
                                rot = qp.tile([P, MC], F32, name="rot",
                                              tag="rot")
                                nc.vector.tensor_copy(rot[0:64, :],
                                                      qh[64:P, :])
                                nc.vector.tensor_copy(rot[64:P, :],
                                                      qh[0:64, :])
                                t1 = qp.tile([P, MC], F32, name="rp1",
                                             tag="rp1")
                                nc.vector.tensor_mul(t1[:], qh[:],
                                                     cosT[:, cs])
                                t2 = qp.tile([P, MC], F32, name="rp2",
                                             tag="rp2")
                                nc.vector.tensor_mul(t2[:], rot[:],
                                                     sinm[:, cs])
                                nc.vector.tensor_add(dsts[hh][:, cs],
                                                     t1[:], t2[:])

        # ============ Phase ATT (own heads, full S, causal skip) ============
        with tc.tile_pool(name="att", bufs=3) as ap, \
             tc.tile_pool(name="attp", bufs=2, space="PSUM") as app:
            for hh in range(HPC):
                for qc in range(4):
                    qs = slice(qc * MC, (qc + 1) * MC)
                    o_ps = app.tile([P, MC], F32, space="PSUM", name="a_po",
                                    tag="a_po")
                    s_ps = app.tile([1, MC], F32, space="PSUM", name="a_ps",
                                    tag="a_ps")
                    njc = 4 * qc + 4
                    for jc in range(njc):
                        pa = app.tile([P, MC], F32, space="PSUM", name="a_pa",
                                      tag="a_pa")
                        nc.tensor.matmul(pa[:],
                                         k_sb[hh][:, jc * P:(jc + 1) * P],
                                         q_sb[hh][:, qs],
                                         start=True, stop=True,
                                         skip_group_check=True)
                        ex = ap.tile([P, MC], BF16, name="a_ex", tag="a_ex")
                        if jc >= 4 * qc:
                            et = ap.tile([P, MC], BF16, name="a_et",
                                         tag="a_et")
                            nc.scalar.activation(et[:], pa[:], AF.Exp,
                                                 scale=SCALE)
                            nc.vector.tensor_mul(ex[:], et[:],
                                                 masks[:, jc - 4 * qc, :])
                        else:
                            nc.scalar.activation(ex[:], pa[:], AF.Exp,
                                                 scale=SCALE)
                        nc.tensor.matmul(s_ps[:], ones_bf[:], ex[:],
                                         start=(jc == 0), stop=(jc == njc - 1),
                                         skip_group_check=True)
                        nc.tensor.matmul(o_ps[:],
                                         v_sb[hh][:, jc, :],
                                         ex[:],
                                         start=(jc == 0), stop=(jc == njc - 1),
                                         skip_group_check=True)
                    rec = ap.tile([1, MC], F32, name="a_rec", tag="a_rec")
                    nc.vector.reciprocal(rec[:], s_ps[:])
                    recb = ap.tile([P, MC], F32, name="a_recb", tag="a_recb")
                    nc.gpsimd.partition_broadcast(recb[:], rec[:])
                    nc.vector.tensor_mul(o_sb[hh][:, qs], o_ps[:], recb[:])
            # ship o to DRAM for AllToAll: rows (j hl p), cols s
            for hh in range(HPC):
                nc.sync.dma_start(
                    out=o_a2a.rearrange("(j hl p) s -> hl p j s",
                                        j=NC, hl=HPC)[hh],
                    in_=o_sb[hh][:])
        esA.close()
        nc.gpsimd.collective_compute("AllToAll", OP.bypass, replica_groups=RG,
                                     ins=[o_a2a[:]], outs=[o_all[:]])

        # ============ Phase OPROJ: x1 = selh + o_all @ ow (local) ============
        esO = ExitStack()
        gpO = esO.enter_context(tc.tile_pool(name="gpO", bufs=1))
        ow_t = [gpO.tile([P, D], BF16, name=f"ow_t{h}") for h in range(H)]
        for h in range(H):
            nc.sync.dma_start(out=ow_t[h][:], in_=ow[h * P:(h + 1) * P, :])
        o_t = [gpO.tile([P, SB], BF16, name=f"o_t{h}") for h in range(H)]
        for h in range(H):
            nc.sync.dma_start(out=o_t[h][:], in_=o_all[h * P:(h + 1) * P, :])
        with tc.tile_pool(name="opjp", bufs=1, space="PSUM") as opp:
            for sc_ in range(2):
                po = [opp.tile([P, MC], F32, space="PSUM", name=f"o_ps{j}",
                               tag=f"o_ps{j}") for j in range(4)]
                for h in range(H):
                    for j in range(4):
                        nc.tensor.matmul(
                            po[j][:], o_t[h][:, sc_ * P:(sc_ + 1) * P],
                            ow_t[h][:, j * MC:(j + 1) * MC],
                            start=(h == 0), stop=(h == H - 1),
                            skip_group_check=True)
                for j in range(4):
                    nc.vector.tensor_add(
                        x1[sc_][:, j * MC:(j + 1) * MC], po[j][:],
                        selh[sc_][:, j * MC:(j + 1) * MC])
        esO.close()

        if phases == "x1":
            with tc.tile_pool(name="xfin", bufs=2) as fp:
                for half in range(2):
                    nc.sync.dma_start(out=upd_out[half * P:(half + 1) * P, :],
                                      in_=x1[half][:])
                    nc.sync.dma_start(
                        out=selidx_out[half * P:(half + 1) * P, :],
                        in_=own_rows[half][:])
                nc.sync.dma_start(out=dbg[:], in_=dbg_t[:])
            return

        # ============ Phase N2: h2 = rmsnorm(x1)*ln2, transposed; AG ========
        with tc.tile_pool(name="n2", bufs=2) as np_, \
             tc.tile_pool(name="n2p", bufs=4, space="PSUM") as npp:
            h2T_own = np_.tile([P, DC, SB], BF16, name="h2T_own")
            for half in range(2):
                h2 = np_.tile([P, D], F32, name="h2")
                _rmsnorm_now(nc, np_, x1[half], h2, epst)
                for d in range(DC):
                    pt = npp.tile([P, P], F32, space="PSUM", name="n2_tp")
                    nc.tensor.transpose(pt[:], h2[:, d * P:(d + 1) * P],
                                        ident[:])
                    nc.scalar.activation(
                        h2T_own[:, d, half * P:(half + 1) * P], pt[:],
                        AF.Copy, scale=lnw_cols[:, DC + d:DC + d + 1])
            for d in range(DC):
                nc.sync.dma_start(out=h2t_in[d * P:(d + 1) * P, :],
                                  in_=h2T_own[:, d, :])
        nc.gpsimd.collective_compute("AllGather", OP.bypass, replica_groups=RG,
                                     ins=[h2t_in[:]], outs=[h2t_all[:]])
        h2t_v = h2t_all.rearrange("(c dd p) s -> dd p c s", c=NC, p=P)

        # ============ Phase MLP ============
        esM = ExitStack()
        gpM = esM.enter_context(tc.tile_pool(name="gpM", bufs=1))
        gw_t = [gpM.tile([P, ICOL], BF16, name=f"gw_t{d}") for d in range(DC)]
        uw_t = [gpM.tile([P, ICOL], BF16, name=f"uw_t{d}") for d in range(DC)]
        for d in range(DC):
            nc.sync.dma_start(out=gw_t[d][:],
                              in_=gatew_s[d * P:(d + 1) * P, :])
            nc.sync.dma_start(out=uw_t[d][:],
                              in_=upw_s[d * P:(d + 1) * P, :])
        act = [gpM.tile([P, S], BF16, name=f"act{ic}")
               for ic in range(len(IC_CH))]
        for half in range(2):
            hs = slice(half * (S // 2), (half + 1) * (S // 2))
            with tc.tile_pool(name="mlpr", bufs=1) as mr, \
                 tc.tile_pool(name="mlp", bufs=2) as mp, \
                 tc.tile_pool(name="mlpp", bufs=2, space="PSUM") as mpp:
                rhs = [mr.tile([P, S // 2], BF16, name=f"m_rhs{d}",
                               tag=f"m_rhs{d}") for d in range(DC)]
                for d in range(DC):
                    nc.sync.dma_start(out=rhs[d][:], in_=h2t_v[d][:, 4 * half:4 * half + 4, :])
                for ic, icw in enumerate(IC_CH):
                    ics = slice(ic * P, ic * P + icw)
                    pg = [mpp.tile([P, MC], F32, space="PSUM",
                                   name=f"m_pg{j}", tag=f"m_pg{j}")
                          for j in range(2)]
                    pu = [mpp.tile([P, MC], F32, space="PSUM",
                                   name=f"m_pu{j}", tag=f"m_pu{j}")
                          for j in range(2)]
                    for d in range(DC):
                        for j in range(2):
                            nc.tensor.matmul(
                                pg[j][:icw, :], gw_t[d][:, ics],
                                rhs[d][:, j * MC:(j + 1) * MC],
                                start=(d == 0), stop=(d == DC - 1),
                                skip_group_check=True)
                    for d in range(DC):
                        for j in range(2):
                            nc.tensor.matmul(
                                pu[j][:icw, :], uw_t[d][:, ics],
                                rhs[d][:, j * MC:(j + 1) * MC],
                                start=(d == 0), stop=(d == DC - 1),
                                skip_group_check=True)
                    for j in range(2):
                        sg = mp.tile([P, MC], F32, name="m_sg", tag="m_sg")
                        nc.scalar.activation(sg[:icw, :], pg[j][:icw, :],
                                             AF.Silu)
                        asl = slice(half * (S // 2) + j * MC,
                                    half * (S // 2) + (j + 1) * MC)
                        nc.vector.tensor_mul(act[ic][:icw, asl],
                                             sg[:icw, :], pu[j][:icw, :])
        # down-proj: stationary act chunks, moving down weights
        with tc.tile_pool(name="dwn", bufs=2) as dp_, \
             tc.tile_pool(name="dwnw", bufs=1) as dw_, \
             tc.tile_pool(name="dwnp", bufs=1, space="PSUM") as dpp:
            dwt = [dw_.tile([P, D], BF16, name=f"dwt{ic}")
                   for ic in range(len(IC_CH))]
            for ic, icw in enumerate(IC_CH):
                nc.sync.dma_start(out=dwt[ic][:icw, :],
                                  in_=downw_s[ic * P:ic * P + icw, :])
            for sc_ in range(SC):
                pd = [dpp.tile([P, MC], F32, space="PSUM", name=f"d_ps{j}",
                               tag=f"d_ps{j}") for j in range(4)]
                for ic, icw in enumerate(IC_CH):
                    for j in range(4):
                        nc.tensor.matmul(
                            pd[j][:],
                            act[ic][:icw, sc_ * P:(sc_ + 1) * P],
                            dwt[ic][:icw, j * MC:(j + 1) * MC],
                            start=(ic == 0), stop=(ic == len(IC_CH) - 1),
                            skip_group_check=True)
                mo = dp_.tile([P, D], BF16, name="d_mo", tag="d_mo")
                for j in range(4):
                    nc.vector.tensor_copy(mo[:, j * MC:(j + 1) * MC],
                                          pd[j][:])
                nc.sync.dma_start(out=mlp_in[sc_ * P:(sc_ + 1) * P, :],
                                  in_=mo[:])
        esM.close()
        nc.gpsimd.collective_compute("ReduceScatter", OP.add,
                                     replica_groups=RG, ins=[mlp_in[:]],
                                     outs=[mlp_rs[:]])

        # ============ Final: gated update ============
        with tc.tile_pool(name="fin", bufs=2) as fp:
            for half in range(2):
                mtb = fp.tile([P, D], BF16, name="f_mtb")
                nc.sync.dma_start(out=mtb[:],
                                  in_=mlp_rs[half * P:(half + 1) * P, :])
                mt = fp.tile([P, D], F32, name="f_mt")
                nc.scalar.activation(mt[:], mtb[:], AF.Copy)
                x2 = fp.tile([P, D], F32, name="f_x2")
                nc.vector.tensor_add(x2[:], x1[half][:], mt[:])
                dlt = fp.tile([P, D], F32, name="f_dlt")
                nc.vector.tensor_sub(dlt[:], x2[:], selh[half][:])
                upd = fp.tile([P, D], F32, name="f_upd")
                nc.vector.scalar_tensor_tensor(
                    upd[:], in0=dlt[:], scalar=gate_g[half][:, :1],
                    in1=selh[half][:], op0=OP.mult, op1=OP.add)
                nc.sync.dma_start(out=upd_out[half * P:(half + 1) * P, :],
                                  in_=upd[:])
                nc.sync.dma_start(out=selidx_out[half * P:(half + 1) * P, :],
                                  in_=own_rows[half][:])
            nc.vector.tensor_copy(dbg_t[:, 8:9], gate_g[0][:])
            nc.sync.dma_start(out=dbg[:], in_=dbg_t[:])


def _row_select_bcast(nc, pool, src_all, col_b, out_bcast):
    """out = broadcast(src_all row-block b), b in {0,1} from col_b."""
    r0 = pool.tile([1, T], F32, name="rs_r0")
    r1 = pool.tile([1, T], F32, name="rs_r1")
    v = src_all.rearrange("(a t) one -> a (t one)", a=2)
    nc.sync.dma_start(out=r0[:], in_=v[0:1, :])
    nc.sync.dma_start(out=r1[:], in_=v[1:2, :])
    b0 = pool.tile([P, T], F32, name="rs_b0")
    b1 = pool.tile([P, T], F32, name="rs_b1")
    nc.gpsimd.partition_broadcast(b0[:], r0[:])
    nc.gpsimd.partition_broadcast(b1[:], r1[:])
    df = pool.tile([P, T], F32, name="rs_df")
    nc.vector.tensor_sub(df[:], b1[:], b0[:])
    nc.vector.scalar_tensor_tensor(out_bcast[:], in0=df[:], scalar=col_b,
                                   in1=b0[:], op0=OP.mult, op1=OP.add)


# =====================================================================
# Host side
# =====================================================================
BF = ml_dtypes.bfloat16


def kernel(**inputs):
    hs = np.asarray(inputs["hidden_states"], np.float32)
    qw = np.asarray(inputs["q_w"], np.float32)
    kw = np.asarray(inputs["k_w"], np.float32)
    vw = np.asarray(inputs["v_w"], np.float32)
    bcu = float(np.asarray(inputs["beta_cu"]))
    bce = float(np.asarray(inputs["beta_ce"]))
    ceo = float(np.asarray(inputs["ce_off"]))

    hs_f = np.ascontiguousarray(hs.reshape(BT, D))
    orig_f = np.asarray(inputs["original"], np.float32).reshape(BT, D)
    post_f = np.asarray(inputs["posterior"], np.float32).reshape(BT, D)
    prior_f = np.asarray(inputs["prior"], np.float32).reshape(BT, D)
    cossin_f = np.ascontiguousarray(np.concatenate(
        [np.asarray(inputs["cos"], np.float32).reshape(BT, HD),
         np.asarray(inputs["sin"], np.float32).reshape(BT, HD)], axis=1))

    ow_bf = np.ascontiguousarray(
        np.asarray(inputs["o_w"], np.float32)).astype(BF)
    gw = np.asarray(inputs["gate_w"], np.float32)
    uw = np.asarray(inputs["up_w"], np.float32)
    dw = np.asarray(inputs["down_w"], np.float32)

    in_maps = []
    for c in range(NC):
        sl = slice(c * TOKS, (c + 1) * TOKS)
        hd_sl = slice(c * HPC * HD, (c + 1) * HPC * HD)
        ic_sl = slice(c * ICOL, (c + 1) * ICOL)
        b = c // 4
        cconst = np.array([[bcu, bce, bce * ceo, c * SB, 0.0,
                            0.0, (c % 4) * TOKS, b]], np.float32)
        in_maps.append({
            "orig_s": np.ascontiguousarray(orig_f[sl]),
            "post_s": np.ascontiguousarray(post_f[sl]),
            "prior_s": np.ascontiguousarray(prior_f[sl]),
            "hidden": hs_f,
            "cossin": cossin_f,
            "qw_s": np.ascontiguousarray(qw[:, hd_sl]).astype(BF),
            "kw_s": np.ascontiguousarray(kw[:, hd_sl]).astype(BF),
            "vw_s": np.ascontiguousarray(vw[:, hd_sl]).astype(BF),
            "qb_s": np.ascontiguousarray(
                np.asarray(inputs["q_b"], np.float32)[hd_sl]).reshape(-1, 1),
            "kb_s": np.ascontiguousarray(
                np.asarray(inputs["k_b"], np.float32)[hd_sl]).reshape(-1, 1),
            "vb_s": np.ascontiguousarray(
                np.asarray(inputs["v_b"], np.float32)[hd_sl]).reshape(-1, 1),
            "ow": ow_bf,
            "ln1w": np.asarray(inputs["ln1_w"], np.float32).reshape(-1, 1),
            "ln2w": np.asarray(inputs["ln2_w"], np.float32).reshape(-1, 1),
            "gatew_s": np.ascontiguousarray(gw[:, ic_sl]).astype(BF),
            "upw_s": np.ascontiguousarray(uw[:, ic_sl]).astype(BF),
            "downw_s": np.ascontiguousarray(dw[ic_sl, :]).astype(BF),
            "cconst": cconst,
        })

    global _last_in_maps
    _last_in_maps = in_maps
    import os
    ph = os.environ.get("KPHASES", "full")
    g1 = os.environ.get("KGATE1", "") == "1"
    key = (ph, g1)
    if key not in _NC_CACHE:
        _NC_CACHE[key] = build(phases=ph, gate1=g1)
    nc = _NC_CACHE[key]
    res = run_bass_kernel_spmd(nc, in_maps, core_ids=list(range(NC)))

    global _last_results
    _last_results = [res.results[c] for c in range(NC)]
    out = hs_f.copy()
    for c in range(NC):
        idx = res.results[c]["selidx_out"][:, 0]
        out[idx] = res.results[c]["upd_out"]
    return out.reshape(B, T, D)


if __name__ == "__main__":
    import reference
    inp = {k: np.asarray(v) for k, v in reference.setup_inputs().items()}
    got = kernel(**inp)
    want = np.asarray(reference.reference(**reference.setup_inputs()))
    err = np.abs(got - want).max() / np.abs(want).max()
    print("rel err:", err)
